# revision 15
# baseline (speedup 1.0000x reference)
"""Multi-head attention (B=4, S=2048, D=1024, H=16, dk=64) on 8 TRN2 NeuronCores.

Sharding: core c = (batch b = c//2, head-group g = c%2 of 8 heads).
Each core computes its head-group's attention output and the partial output
projection (Wo rows for its heads); the host sums the two partials per batch
and adds the (folded) output bias.

Per-core math, all in fp32r matmuls (fp32 storage, full-rate reduced-precision
multiply; PSUM accumulation fp32):
  QT = (Wq/8)^T X_q^T + bq/8      [512, 2048]  (heads stacked on partitions)
  KT = Wk^T X_k^T + bk            [512, 2048]
  V  = X_v Wv                     [2048, 512]  + ones column per head (aug)
  per head-pair hp = (h0, h1), Sq-block j:
    S^T_i pair packed in PE row groups 0/64 -> one 2-bank PSUM [128, 1024]
    E_i = exp(S^T pair)           one ScalarE op over 1024 (no max-subtraction
                                   needed: scores ~ N(0,1), max < ~6)
    [U^T; r] += [V_i | 1]^T E_i   [65, 512] per head (row 64 = denominators)
    O^T = U^T * (1/r)             (DVE reciprocal_approx_fast + PE outer bcast)
  Y^T_partial = Wo_c^T O^T        [1024, 2048]
V-bias and output bias are folded on the host: softmax rows sum to 1, so
bv contributes bv_cat @ Wo + bo to every row.

Perf notes (HW-measured):
  - f32r matmul N=512 runs at 227 ns (full rate) incl. overlapped LDWEIGHTS.
  - K=64 matmuls on a single row group run at half rate; alternating row
    groups (base partitions 0/64) makes the pair run concurrently.
  - A [128,1024] 2-bank PSUM tile lets one EXP drain a whole score pair.
"""

import numpy as np

B, S, D = 4, 2048, 1024
H, DK = 16, 64
LH = 8                 # heads per core
HK = LH * DK           # 512 (local concat dim)
BLK = 512              # Sq block size
NB = S // BLK          # 4
ST = S // 128          # 16 Skv tiles
KT = D // 128          # 8 contraction tiles over D
MT = HK // 128         # 4 m-tiles over local heads

_CACHE = {}


def _build_program():
    from contextlib import ExitStack
    import concourse.bass as bass
    import concourse.tile as tile
    from concourse import bacc, mybir

    f32 = mybir.dt.float32
    f32r = mybir.dt.float32r
    bf16 = mybir.dt.bfloat16
    u16 = mybir.dt.uint16
    u32 = mybir.dt.uint32
    Exp = mybir.ActivationFunctionType.Exp

    nc = bacc.Bacc("TRN2", target_bir_lowering=False, debug=False, num_devices=8)

    xq_d = nc.dram_tensor("xq_t", [D, S], f32r, kind="ExternalInput")
    xk_d = nc.dram_tensor("xk_t", [D, S], f32r, kind="ExternalInput")
    xv_d = nc.dram_tensor("xv_t", [D, S], f32r, kind="ExternalInput")
    wq_d = nc.dram_tensor("wq", [D, HK], f32r, kind="ExternalInput")
    wk_d = nc.dram_tensor("wk", [D, HK], f32r, kind="ExternalInput")
    wv_d = nc.dram_tensor("wv", [D, HK], f32r, kind="ExternalInput")
    wo_d = nc.dram_tensor("wo", [HK, D], f32r, kind="ExternalInput")
    bq_d = nc.dram_tensor("bq2", [128, MT], f32, kind="ExternalInput")
    bk_d = nc.dram_tensor("bk2", [128, MT], f32, kind="ExternalInput")
    y_d = nc.dram_tensor("y_t", [D, S], f32, kind="ExternalOutput")

    with tile.TileContext(nc) as tc, ExitStack() as ctx:
        wpool = ctx.enter_context(tc.tile_pool(name="w", bufs=2))
        big = ctx.enter_context(tc.tile_pool(name="big", bufs=1))
        xs = ctx.enter_context(tc.tile_pool(name="xs", bufs=6))
        es_pool = ctx.enter_context(tc.tile_pool(name="es", bufs=8))
        ot_pool = ctx.enter_context(tc.tile_pool(name="ot", bufs=2))
        rpool = ctx.enter_context(tc.tile_pool(name="r", bufs=3))
        upool = ctx.enter_context(tc.tile_pool(name="u", bufs=3))
        ypool = ctx.enter_context(tc.tile_pool(name="y", bufs=3))
        # PSUM: psS 2x[128,1024] (4 banks) + psU 4x[65,512] = 8
        psS = ctx.enter_context(tc.tile_pool(name="psS", bufs=2, space="PSUM"))
        psU = ctx.enter_context(tc.tile_pool(name="psU", bufs=4, space="PSUM"))

        bq_sb = big.tile([128, MT], f32)
        bk_sb = big.tile([128, MT], f32)
        nc.sync.dma_start(bq_sb[:], bq_d[:])
        nc.sync.dma_start(bk_sb[:], bk_d[:])
        qt = big.tile([128, MT, S], f32r)
        kt_ = big.tile([128, MT, S], f32r)
        # V and exp(S) run in bf16: the attention-weight x V product tolerates
        # bf16 (measured 2.2e-3 end-to-end) and bf16 weight loads overlap
        # matmuls (FWL + background weight buffer), unlike f32r ones.
        vaug = big.tile([128, ST, LH, DK + 1], bf16)
        # flat memset (strided 4D memset is invalid ISA); V-proj drains
        # overwrite cols 0:DK per head, leaving the aug ones-column intact
        nc.vector.memset(vaug[:, :, :, :].bitcast(u16), 0x3F80)

        def proj_qk(x_dram, w_dram, bias_sb, dst, wname):
            # dst[p, mt, s] = sum_d w[d, mt*128+p] * x^T[d, s] + bias
            # mt pairs share one 2-bank PSUM tile (left/right halves)
            w_sb = wpool.tile([128, KT, HK], f32r, tag="w", name=f"w_{wname}")
            nc.sync.dma_start(w_sb[:], w_dram.ap().rearrange("(kt p) m -> p kt m", p=128))
            for j in range(NB):
                pp = [psS.tile([128, 2 * BLK], f32, tag="psS", name=f"pp_{wname}{j}_{t}")
                      for t in range(2)]
                for kt in range(KT):
                    xt = xs.tile([128, BLK], f32r, tag="xs", name=f"xt_{wname}{j}_{kt}")
                    # alternate DMA issue engines for queue parallelism
                    eng = nc.sync if kt % 2 == 0 else nc.gpsimd
                    eng.dma_start(
                        xt[:], x_dram[kt * 128 : (kt + 1) * 128, j * BLK : (j + 1) * BLK]
                    )
                    for mt in range(MT):
                        half = (mt % 2) * BLK
                        nc.tensor.matmul(
                            pp[mt // 2][:, half : half + BLK],
                            w_sb[:, kt, mt * 128 : (mt + 1) * 128],
                            xt[:],
                            start=(kt == 0),
                            stop=(kt == KT - 1),
                            skip_group_check=True,
                        )
                for mt in range(MT):
                    half = (mt % 2) * BLK
                    nc.vector.tensor_scalar_add(
                        dst[:, mt, j * BLK : (j + 1) * BLK],
                        pp[mt // 2][:, half : half + BLK],
                        bias_sb[:, mt : mt + 1],
                    )

        # V projection: V[st*128+p, h*64+k] per Skv tile st, drained into vaug
        wv_sb = wpool.tile([128, KT, HK], f32r, tag="w")
        nc.sync.dma_start(wv_sb[:], wv_d.ap().rearrange("(kt p) m -> p kt m", p=128))
        for j in range(NB):
            pp = [psU.tile([128, BLK], f32, tag="psU", name=f"pp_v{j}_{t}")
                  for t in range(4)]
            for kt in range(KT):
                xt = xs.tile([128, BLK], f32r, tag="xs", name=f"xt_v{j}_{kt}")
                eng = nc.sync if kt % 2 == 0 else nc.gpsimd
                eng.dma_start(
                    xt[:], xv_d[kt * 128 : (kt + 1) * 128, j * BLK : (j + 1) * BLK]
                )
                for q in range(4):
                    nc.tensor.matmul(
                        pp[q][:],
                        xt[:, q * 128 : (q + 1) * 128],
                        wv_sb[:, kt, :],
                        start=(kt == 0),
                        stop=(kt == KT - 1),
                        skip_group_check=True,
                    )
            for q in range(4):
                st = j * 4 + q
                nc.vector.tensor_copy(
                    vaug[:, st, :, 0:DK],
                    pp[q][:].rearrange("p (h k) -> p h k", h=LH),
                )

        proj_qk(xk_d, wk_d, bk_sb, kt_, "k")
        proj_qk(xq_d, wq_d, bq_sb, qt, "q")

        # Attention + output projection, fused per Sq-block
        wo_sb = wpool.tile([128, MT, D], f32r, tag="w")  # same slot bytes as proj weights
        nc.sync.dma_start(
            wo_sb[:], wo_d.ap().rearrange("(kt p) m -> p kt m", p=128)
        )
        def emit_outproj(jprev, ot_prev, mos):
            # output projection for block jprev, selected mo tiles; psy from the
            # psU pool so the scores (psS) pipeline keeps flowing
            for mo in mos:
                psy = psU.tile([128, BLK], f32, tag="psU", name=f"psy{jprev}_{mo}")
                for kt in range(MT):
                    nc.tensor.matmul(
                        psy[:],
                        wo_sb[:, kt, mo * 128 : (mo + 1) * 128],
                        ot_prev[:, kt, :],
                        start=(kt == 0),
                        stop=(kt == MT - 1),
                        skip_group_check=True,
                    )
                ysb = ypool.tile([128, BLK], f32, tag="y", name=f"ysb{jprev}_{mo}")
                nc.vector.tensor_copy(ysb[:], psy[:])
                nc.sync.dma_start(
                    y_d[mo * 128 : (mo + 1) * 128,
                        jprev * BLK : (jprev + 1) * BLK], ysb[:]
                )

        ot_prev = None
        for j in range(NB):
            otj = ot_pool.tile([128, MT, BLK], f32r)
            for hp in range(LH // 2):
                mt = hp
                psu = [psU.tile([DK + 1, BLK], f32, tag="psU",
                                name=f"ps_u{j}_{hp}_{p2}") for p2 in range(2)]
                for i in range(ST):
                    ps2 = psS.tile([128, 2 * BLK], f32, tag="psS",
                                   name=f"ps_s{j}_{hp}_{i}")
                    for pi in range(2):
                        bp = pi * 64
                        nc.tensor.matmul(
                            ps2[:, pi * BLK : (pi + 1) * BLK],
                            kt_[bp : bp + 64, mt, i * 128 : (i + 1) * 128],
                            qt[bp : bp + 64, mt, j * BLK : (j + 1) * BLK],
                            start=True,
                            stop=True,
                            skip_group_check=True,
                        )
                    es = es_pool.tile([128, 2 * BLK], bf16, tag="es")
                    nc.scalar.activation(es[:], ps2[:], Exp)
                    for pi in range(2):
                        h = 2 * hp + pi
                        nc.tensor.matmul(
                            psu[pi][:],
                            vaug[:, i, h, :],
                            es[:, pi * BLK : (pi + 1) * BLK],
                            start=(i == 0),
                            stop=(i == ST - 1),
                            skip_group_check=True,
                        )
                for pi in range(2):
                    bp = pi * 64
                    # r row (psum partition 64) -> partition 0 (plain DVE copy
                    # handles the shift; the custom reciprocal op does not, so
                    # it runs after, aligned at partition 0), then broadcast
                    # 1/r across 64 partitions on the idle GpSimd engine.
                    rrow = rpool.tile([1, BLK], f32, tag="r", name=f"rr{j}_{hp}_{pi}")
                    nc.vector.tensor_copy(rrow[:], psu[pi][DK : DK + 1, :])
                    rf = rpool.tile([1, BLK], f32, tag="rf", name=f"rf{j}_{hp}_{pi}")
                    nc.vector.reciprocal_approx_fast(rf[:], rrow[:])
                    rbc = upool.tile([DK, BLK], f32, tag="rb", name=f"rb{j}_{hp}_{pi}")
                    nc.gpsimd.partition_broadcast(rbc[:], rf[:])
                    nc.vector.tensor_mul(otj[bp : bp + 64, mt, :],
                                         psu[pi][0:DK, :], rbc[:])
                if hp == 0 and ot_prev is not None:
                    emit_outproj(j - 1, ot_prev, range(KT))
            ot_prev = otj
        emit_outproj(NB - 1, ot_prev, range(KT))

    nc.compile()
    return nc


def get_program():
    if "nc" not in _CACHE:
        _CACHE["nc"] = _build_program()
    return _CACHE["nc"]


def make_core_inputs(query, key, value, Wq, bq, Wk, bk, Wv, bv, Wo, bo):
    """Build the 8 per-core input dicts (and the folded output bias)."""
    f = np.float32
    in_maps = []
    for c in range(8):
        b, g = c // 2, c % 2
        hs = slice(g * LH, (g + 1) * LH)
        m = {
            "xq_t": np.ascontiguousarray(query[b].T, dtype=f),
            "xk_t": np.ascontiguousarray(key[b].T, dtype=f),
            "xv_t": np.ascontiguousarray(value[b].T, dtype=f),
            "wq": np.ascontiguousarray(
                Wq[hs].transpose(1, 0, 2).reshape(D, HK) / 8.0, dtype=f
            ),
            "wk": np.ascontiguousarray(
                Wk[hs].transpose(1, 0, 2).reshape(D, HK), dtype=f
            ),
            "wv": np.ascontiguousarray(
                Wv[hs].transpose(1, 0, 2).reshape(D, HK), dtype=f
            ),
            "wo": np.ascontiguousarray(Wo[g * HK : (g + 1) * HK, :], dtype=f),
            "bq2": np.ascontiguousarray(
                (bq[hs].reshape(HK) / 8.0).reshape(MT, 128).T, dtype=f
            ),
            "bk2": np.ascontiguousarray(
                bk[hs].reshape(HK).reshape(MT, 128).T, dtype=f
            ),
        }
        in_maps.append(m)
    bo_eff = (bv.reshape(H * DK).astype(np.float64) @ Wo.astype(np.float64)
              + bo.astype(np.float64)).astype(f)
    return in_maps, bo_eff


def combine_outputs(results, bo_eff):
    """results: list of 8 dicts with 'y_t' [D, S]. Returns [B, S, D] f32."""
    out = np.empty((B, S, D), dtype=np.float32)
    for b in range(B):
        acc = results[2 * b]["y_t"] + results[2 * b + 1]["y_t"]
        out[b] = acc.T + bo_eff[None, :]
    return out


def kernel(**inputs):
    from concourse.bass_utils import run_bass_kernel_spmd

    inputs = {k: np.asarray(v) for k, v in inputs.items()}
    nc = get_program()
    in_maps, bo_eff = make_core_inputs(
        inputs["query"], inputs["key"], inputs["value"],
        inputs["Wq"], inputs["bq"], inputs["Wk"], inputs["bk"],
        inputs["Wv"], inputs["bv"], inputs["Wo"], inputs["bo"],
    )
    res = run_bass_kernel_spmd(nc, in_maps, list(range(8)))
    return combine_outputs(res.results, bo_eff)


# revision 16
# speedup vs baseline: 1.0746x; 1.0746x over previous
"""Multi-head attention (B=4, S=2048, D=1024, H=16, dk=64) on 8 TRN2 NeuronCores.

Sharding: core c = (batch b = c//2, head-group g = c%2 of 8 heads).
Each core computes its head-group's attention output and the partial output
projection (Wo rows for its heads); the host sums the two partials per batch
and adds the (folded) output bias.

Per-core math, all in fp32r matmuls (fp32 storage, full-rate reduced-precision
multiply; PSUM accumulation fp32):
  QT = (Wq/8)^T X_q^T + bq/8      [512, 2048]  (heads stacked on partitions)
  KT = Wk^T X_k^T + bk            [512, 2048]
  V  = X_v Wv                     [2048, 512]  + ones column per head (aug)
  per head-pair hp = (h0, h1), Sq-block j:
    S^T_i pair packed in PE row groups 0/64 -> one 2-bank PSUM [128, 1024]
    E_i = exp(S^T pair)           one ScalarE op over 1024 (no max-subtraction
                                   needed: scores ~ N(0,1), max < ~6)
    [U^T; r] += [V_i | 1]^T E_i   [65, 512] per head (row 64 = denominators)
    O^T = U^T * (1/r)             (DVE reciprocal_approx_fast + PE outer bcast)
  Y^T_partial = Wo_c^T O^T        [1024, 2048]
V-bias and output bias are folded on the host: softmax rows sum to 1, so
bv contributes bv_cat @ Wo + bo to every row.

Perf notes (HW-measured):
  - f32r matmul N=512 runs at 227 ns (full rate) incl. overlapped LDWEIGHTS.
  - K=64 matmuls on a single row group run at half rate; alternating row
    groups (base partitions 0/64) makes the pair run concurrently.
  - A [128,1024] 2-bank PSUM tile lets one EXP drain a whole score pair.
"""

import numpy as np

B, S, D = 4, 2048, 1024
H, DK = 16, 64
LH = 8                 # heads per core
HK = LH * DK           # 512 (local concat dim)
BLK = 512              # Sq block size
NB = S // BLK          # 4
ST = S // 128          # 16 Skv tiles
KT = D // 128          # 8 contraction tiles over D
MT = HK // 128         # 4 m-tiles over local heads

_CACHE = {}


def _build_program():
    from contextlib import ExitStack
    import concourse.bass as bass
    import concourse.tile as tile
    from concourse import bacc, mybir

    f32 = mybir.dt.float32
    f32r = mybir.dt.float32r
    bf16 = mybir.dt.bfloat16
    u16 = mybir.dt.uint16
    u32 = mybir.dt.uint32
    Exp = mybir.ActivationFunctionType.Exp

    nc = bacc.Bacc("TRN2", target_bir_lowering=False, debug=False, num_devices=8)

    xq_d = nc.dram_tensor("xq_t", [D, S], f32r, kind="ExternalInput")
    xk_d = nc.dram_tensor("xk_t", [D, S], f32r, kind="ExternalInput")
    xv_d = nc.dram_tensor("xv_t", [D, S], f32r, kind="ExternalInput")
    wq_d = nc.dram_tensor("wq", [D, HK], f32r, kind="ExternalInput")
    wk_d = nc.dram_tensor("wk", [D, HK], f32r, kind="ExternalInput")
    wv_d = nc.dram_tensor("wv", [D, HK], f32r, kind="ExternalInput")
    wo_d = nc.dram_tensor("wo", [HK, D], f32r, kind="ExternalInput")
    bq_d = nc.dram_tensor("bq2", [128, MT], f32, kind="ExternalInput")
    bk_d = nc.dram_tensor("bk2", [128, MT], f32, kind="ExternalInput")
    y_d = nc.dram_tensor("y_t", [D, S], f32, kind="ExternalOutput")

    with tile.TileContext(nc) as tc, ExitStack() as ctx:
        wpool = ctx.enter_context(tc.tile_pool(name="w", bufs=2))
        big = ctx.enter_context(tc.tile_pool(name="big", bufs=1))
        xs = ctx.enter_context(tc.tile_pool(name="xs", bufs=2))
        es_pool = ctx.enter_context(tc.tile_pool(name="es", bufs=6))
        ot_pool = ctx.enter_context(tc.tile_pool(name="ot", bufs=2))
        rpool = ctx.enter_context(tc.tile_pool(name="r", bufs=2))
        upool = ctx.enter_context(tc.tile_pool(name="u", bufs=2))
        ypool = ctx.enter_context(tc.tile_pool(name="y", bufs=2))
        # PSUM: psS 2x[128,1024] (4 banks) + psU 4x[65,512] = 8
        psS = ctx.enter_context(tc.tile_pool(name="psS", bufs=2, space="PSUM"))
        psU = ctx.enter_context(tc.tile_pool(name="psU", bufs=4, space="PSUM"))

        bq_sb = big.tile([128, MT], f32)
        bk_sb = big.tile([128, MT], f32)
        nc.sync.dma_start(bq_sb[:], bq_d[:])
        nc.sync.dma_start(bk_sb[:], bk_d[:])
        qt = big.tile([128, MT, S], f32r)
        kt_ = big.tile([128, MT, S], f32r)
        # V and exp(S) run in bf16: the attention-weight x V product tolerates
        # bf16 (measured 2.2e-3 end-to-end) and bf16 weight loads overlap
        # matmuls (FWL + background weight buffer), unlike f32r ones.
        vaug = big.tile([128, ST, LH, DK + 1], bf16)
        # flat memset (strided 4D memset is invalid ISA); V-proj drains
        # overwrite cols 0:DK per head, leaving the aug ones-column intact
        nc.vector.memset(vaug[:, :, :, :].bitcast(u16), 0x3F80)

        def proj_qk(x_dram, w_dram, bias_sb, dst, wname):
            # dst[p, mt, s] = sum_d w[d, mt*128+p] * x^T[d, s] + bias
            # mt pairs share one 2-bank PSUM tile (left/right halves)
            w_sb = wpool.tile([128, KT, HK], f32r, tag="w", name=f"w_{wname}")
            nc.sync.dma_start(w_sb[:], w_dram.ap().rearrange("(kt p) m -> p kt m", p=128))
            for j in range(NB):
                pp = [psS.tile([128, 2 * BLK], f32, tag="psS", name=f"pp_{wname}{j}_{t}")
                      for t in range(2)]
                # one 2 MiB DMA per block streams at full HBM bandwidth
                xt = xs.tile([128, KT, BLK], f32r, tag="xs", name=f"xt_{wname}{j}")
                nc.sync.dma_start(
                    xt[:],
                    x_dram.ap()[:, j * BLK : (j + 1) * BLK]
                    .rearrange("(kt p) s -> p kt s", p=128),
                )
                for kt in range(KT):
                    for mt in range(MT):
                        half = (mt % 2) * BLK
                        nc.tensor.matmul(
                            pp[mt // 2][:, half : half + BLK],
                            w_sb[:, kt, mt * 128 : (mt + 1) * 128],
                            xt[:, kt, :],
                            start=(kt == 0),
                            stop=(kt == KT - 1),
                            skip_group_check=True,
                        )
                for mt in range(MT):
                    half = (mt % 2) * BLK
                    nc.vector.tensor_scalar_add(
                        dst[:, mt, j * BLK : (j + 1) * BLK],
                        pp[mt // 2][:, half : half + BLK],
                        bias_sb[:, mt : mt + 1],
                    )

        proj_qk(xk_d, wk_d, bk_sb, kt_, "k")
        proj_qk(xq_d, wq_d, bq_sb, qt, "q")

        # V projection: V[st*128+p, h*64+k] per Skv tile st, drained into vaug
        wv_sb = wpool.tile([128, KT, HK], f32r, tag="w")
        nc.sync.dma_start(wv_sb[:], wv_d.ap().rearrange("(kt p) m -> p kt m", p=128))
        for j in range(NB):
            pp = [psU.tile([128, BLK], f32, tag="psU", name=f"pp_v{j}_{t}")
                  for t in range(4)]
            xt = xs.tile([128, KT, BLK], f32r, tag="xs", name=f"xt_v{j}")
            nc.sync.dma_start(
                xt[:],
                xv_d.ap()[:, j * BLK : (j + 1) * BLK]
                .rearrange("(kt p) s -> p kt s", p=128),
            )
            for kt in range(KT):
                for q in range(4):
                    nc.tensor.matmul(
                        pp[q][:],
                        xt[:, kt, q * 128 : (q + 1) * 128],
                        wv_sb[:, kt, :],
                        start=(kt == 0),
                        stop=(kt == KT - 1),
                        skip_group_check=True,
                    )
            for q in range(4):
                st = j * 4 + q
                nc.vector.tensor_copy(
                    vaug[:, st, :, 0:DK],
                    pp[q][:].rearrange("p (h k) -> p h k", h=LH),
                )

        # Attention + output projection, fused per Sq-block
        wo_sb = wpool.tile([128, MT, D], f32r, tag="w")  # same slot bytes as proj weights
        nc.sync.dma_start(
            wo_sb[:], wo_d.ap().rearrange("(kt p) m -> p kt m", p=128)
        )
        def emit_outproj(jprev, ot_prev, mos):
            # output projection for block jprev, selected mo tiles; psy from the
            # psU pool so the scores (psS) pipeline keeps flowing
            for mo in mos:
                psy = psU.tile([128, BLK], f32, tag="psU", name=f"psy{jprev}_{mo}")
                for kt in range(MT):
                    nc.tensor.matmul(
                        psy[:],
                        wo_sb[:, kt, mo * 128 : (mo + 1) * 128],
                        ot_prev[:, kt, :],
                        start=(kt == 0),
                        stop=(kt == MT - 1),
                        skip_group_check=True,
                    )
                ysb = ypool.tile([128, BLK], f32, tag="y", name=f"ysb{jprev}_{mo}")
                nc.vector.tensor_copy(ysb[:], psy[:])
                nc.sync.dma_start(
                    y_d[mo * 128 : (mo + 1) * 128,
                        jprev * BLK : (jprev + 1) * BLK], ysb[:]
                )

        ot_prev = None
        for j in range(NB):
            otj = ot_pool.tile([128, MT, BLK], f32r)
            for hp in range(LH // 2):
                mt = hp
                psu = [psU.tile([DK + 1, BLK], f32, tag="psU",
                                name=f"ps_u{j}_{hp}_{p2}") for p2 in range(2)]
                for i in range(ST):
                    ps2 = psS.tile([128, 2 * BLK], f32, tag="psS",
                                   name=f"ps_s{j}_{hp}_{i}")
                    for pi in range(2):
                        bp = pi * 64
                        nc.tensor.matmul(
                            ps2[:, pi * BLK : (pi + 1) * BLK],
                            kt_[bp : bp + 64, mt, i * 128 : (i + 1) * 128],
                            qt[bp : bp + 64, mt, j * BLK : (j + 1) * BLK],
                            start=True,
                            stop=True,
                            skip_group_check=True,
                        )
                    es = es_pool.tile([128, 2 * BLK], bf16, tag="es")
                    nc.scalar.activation(es[:], ps2[:], Exp)
                    for pi in range(2):
                        h = 2 * hp + pi
                        nc.tensor.matmul(
                            psu[pi][:],
                            vaug[:, i, h, :],
                            es[:, pi * BLK : (pi + 1) * BLK],
                            start=(i == 0),
                            stop=(i == ST - 1),
                            skip_group_check=True,
                        )
                for pi in range(2):
                    bp = pi * 64
                    # r row (psum partition 64) -> partition 0 (plain DVE copy
                    # handles the shift; the custom reciprocal op does not, so
                    # it runs after, aligned at partition 0), then broadcast
                    # 1/r across 64 partitions on the idle GpSimd engine.
                    rrow = rpool.tile([1, BLK], f32, tag="r", name=f"rr{j}_{hp}_{pi}")
                    nc.vector.tensor_copy(rrow[:], psu[pi][DK : DK + 1, :])
                    rf = rpool.tile([1, BLK], f32, tag="rf", name=f"rf{j}_{hp}_{pi}")
                    nc.vector.reciprocal_approx_fast(rf[:], rrow[:])
                    rbc = upool.tile([DK, BLK], f32, tag="rb", name=f"rb{j}_{hp}_{pi}")
                    nc.gpsimd.partition_broadcast(rbc[:], rf[:])
                    nc.vector.tensor_mul(otj[bp : bp + 64, mt, :],
                                         psu[pi][0:DK, :], rbc[:])
                if hp == 0 and ot_prev is not None:
                    emit_outproj(j - 1, ot_prev, range(KT))
            ot_prev = otj
        emit_outproj(NB - 1, ot_prev, range(KT))

    nc.compile()
    return nc


def get_program():
    if "nc" not in _CACHE:
        _CACHE["nc"] = _build_program()
    return _CACHE["nc"]


def make_core_inputs(query, key, value, Wq, bq, Wk, bk, Wv, bv, Wo, bo):
    """Build the 8 per-core input dicts (and the folded output bias)."""
    f = np.float32
    in_maps = []
    for c in range(8):
        b, g = c // 2, c % 2
        hs = slice(g * LH, (g + 1) * LH)
        m = {
            "xq_t": np.ascontiguousarray(query[b].T, dtype=f),
            "xk_t": np.ascontiguousarray(key[b].T, dtype=f),
            "xv_t": np.ascontiguousarray(value[b].T, dtype=f),
            "wq": np.ascontiguousarray(
                Wq[hs].transpose(1, 0, 2).reshape(D, HK) / 8.0, dtype=f
            ),
            "wk": np.ascontiguousarray(
                Wk[hs].transpose(1, 0, 2).reshape(D, HK), dtype=f
            ),
            "wv": np.ascontiguousarray(
                Wv[hs].transpose(1, 0, 2).reshape(D, HK), dtype=f
            ),
            "wo": np.ascontiguousarray(Wo[g * HK : (g + 1) * HK, :], dtype=f),
            "bq2": np.ascontiguousarray(
                (bq[hs].reshape(HK) / 8.0).reshape(MT, 128).T, dtype=f
            ),
            "bk2": np.ascontiguousarray(
                bk[hs].reshape(HK).reshape(MT, 128).T, dtype=f
            ),
        }
        in_maps.append(m)
    bo_eff = (bv.reshape(H * DK).astype(np.float64) @ Wo.astype(np.float64)
              + bo.astype(np.float64)).astype(f)
    return in_maps, bo_eff


def combine_outputs(results, bo_eff):
    """results: list of 8 dicts with 'y_t' [D, S]. Returns [B, S, D] f32."""
    out = np.empty((B, S, D), dtype=np.float32)
    for b in range(B):
        acc = results[2 * b]["y_t"] + results[2 * b + 1]["y_t"]
        out[b] = acc.T + bo_eff[None, :]
    return out


def kernel(**inputs):
    from concourse.bass_utils import run_bass_kernel_spmd

    inputs = {k: np.asarray(v) for k, v in inputs.items()}
    nc = get_program()
    in_maps, bo_eff = make_core_inputs(
        inputs["query"], inputs["key"], inputs["value"],
        inputs["Wq"], inputs["bq"], inputs["Wk"], inputs["bk"],
        inputs["Wv"], inputs["bv"], inputs["Wo"], inputs["bo"],
    )
    res = run_bass_kernel_spmd(nc, in_maps, list(range(8)))
    return combine_outputs(res.results, bo_eff)


# revision 17
# speedup vs baseline: 1.1039x; 1.0273x over previous
"""Multi-head attention (B=4, S=2048, D=1024, H=16, dk=64) on 8 TRN2 NeuronCores.

Sharding: core c = (batch b = c//2, head-group g = c%2 of 8 heads).
Each core computes its head-group's attention output and the partial output
projection (Wo rows for its heads); the host sums the two partials per batch
and adds the (folded) output bias.

Per-core math, all in fp32r matmuls (fp32 storage, full-rate reduced-precision
multiply; PSUM accumulation fp32):
  QT = (Wq/8)^T X_q^T + bq/8      [512, 2048]  (heads stacked on partitions)
  KT = Wk^T X_k^T + bk            [512, 2048]
  V  = X_v Wv                     [2048, 512]  + ones column per head (aug)
  per head-pair hp = (h0, h1), Sq-block j:
    S^T_i pair packed in PE row groups 0/64 -> one 2-bank PSUM [128, 1024]
    E_i = exp(S^T pair)           one ScalarE op over 1024 (no max-subtraction
                                   needed: scores ~ N(0,1), max < ~6)
    [U^T; r] += [V_i | 1]^T E_i   [65, 512] per head (row 64 = denominators)
    O^T = U^T * (1/r)             (DVE reciprocal_approx_fast + PE outer bcast)
  Y^T_partial = Wo_c^T O^T        [1024, 2048]
V-bias and output bias are folded on the host: softmax rows sum to 1, so
bv contributes bv_cat @ Wo + bo to every row.

Perf notes (HW-measured):
  - f32r matmul N=512 runs at 227 ns (full rate) incl. overlapped LDWEIGHTS.
  - K=64 matmuls on a single row group run at half rate; alternating row
    groups (base partitions 0/64) makes the pair run concurrently.
  - A [128,1024] 2-bank PSUM tile lets one EXP drain a whole score pair.
"""

import numpy as np

B, S, D = 4, 2048, 1024
H, DK = 16, 64
LH = 8                 # heads per core
HK = LH * DK           # 512 (local concat dim)
BLK = 512              # Sq block size
NB = S // BLK          # 4
ST = S // 128          # 16 Skv tiles
KT = D // 128          # 8 contraction tiles over D
MT = HK // 128         # 4 m-tiles over local heads

_CACHE = {}


def _build_program():
    from contextlib import ExitStack
    import concourse.bass as bass
    import concourse.tile as tile
    from concourse import bacc, mybir

    f32 = mybir.dt.float32
    f32r = mybir.dt.float32r
    bf16 = mybir.dt.bfloat16
    u16 = mybir.dt.uint16
    u32 = mybir.dt.uint32
    Exp = mybir.ActivationFunctionType.Exp

    nc = bacc.Bacc("TRN2", target_bir_lowering=False, debug=False, num_devices=8)

    xq_d = nc.dram_tensor("xq_t", [D, S], f32r, kind="ExternalInput")
    xk_d = nc.dram_tensor("xk_t", [D, S], f32r, kind="ExternalInput")
    xv_d = nc.dram_tensor("xv_t", [D, S], f32r, kind="ExternalInput")
    wq_d = nc.dram_tensor("wq", [D, HK], f32r, kind="ExternalInput")
    wk_d = nc.dram_tensor("wk", [D, HK], f32r, kind="ExternalInput")
    wv_d = nc.dram_tensor("wv", [D, HK], f32r, kind="ExternalInput")
    wo_d = nc.dram_tensor("wo", [HK, D], f32r, kind="ExternalInput")
    bq_d = nc.dram_tensor("bq2", [128, MT], f32, kind="ExternalInput")
    bk_d = nc.dram_tensor("bk2", [128, MT], f32, kind="ExternalInput")
    y_d = nc.dram_tensor("y_t", [D, S], f32, kind="ExternalOutput")

    with tile.TileContext(nc) as tc, ExitStack() as ctx:
        wpool = ctx.enter_context(tc.tile_pool(name="w", bufs=2))
        big = ctx.enter_context(tc.tile_pool(name="big", bufs=1))
        xs = ctx.enter_context(tc.tile_pool(name="xs", bufs=2))
        es_pool = ctx.enter_context(tc.tile_pool(name="es", bufs=6))
        ot_pool = ctx.enter_context(tc.tile_pool(name="ot", bufs=2))
        rpool = ctx.enter_context(tc.tile_pool(name="r", bufs=2))
        upool = ctx.enter_context(tc.tile_pool(name="u", bufs=2))
        ypool = ctx.enter_context(tc.tile_pool(name="y", bufs=2))
        # PSUM: psS 2x[128,1024] (4 banks) + psU 4x[65,512] = 8
        psS = ctx.enter_context(tc.tile_pool(name="psS", bufs=2, space="PSUM"))
        psU = ctx.enter_context(tc.tile_pool(name="psU", bufs=4, space="PSUM"))

        bq_sb = big.tile([128, MT], f32)
        bk_sb = big.tile([128, MT], f32)
        nc.sync.dma_start(bq_sb[:], bq_d[:])
        nc.sync.dma_start(bk_sb[:], bk_d[:])
        qt = big.tile([128, MT, S], f32r)
        kt_ = big.tile([128, MT, S], f32r)
        # V and exp(S) run in bf16: the attention-weight x V product tolerates
        # bf16 (measured 2.2e-3 end-to-end) and bf16 weight loads overlap
        # matmuls (FWL + background weight buffer), unlike f32r ones.
        vaug = big.tile([128, ST, LH, DK + 1], bf16)
        # flat memset (strided 4D memset is invalid ISA); V-proj drains
        # overwrite cols 0:DK per head, leaving the aug ones-column intact
        nc.vector.memset(vaug[:, :, :, :].bitcast(u16), 0x3F80)

        def proj_qk(x_dram, w_dram, bias_sb, dst, wname):
            # dst[p, mt, s] = sum_d w[d, mt*128+p] * x^T[d, s] + bias
            # mt pairs share one 2-bank PSUM tile (left/right halves)
            w_sb = wpool.tile([128, KT, HK], f32r, tag="w", name=f"w_{wname}")
            nc.sync.dma_start(w_sb[:], w_dram.ap().rearrange("(kt p) m -> p kt m", p=128))
            for j in range(NB):
                # alternate PSUM pools so consecutive block-rounds pipeline
                if j % 2 == 0:
                    pp2 = [psS.tile([128, 2 * BLK], f32, tag="psS",
                                    name=f"pp_{wname}{j}_{t}") for t in range(2)]
                    pp = [pp2[mt // 2][:, (mt % 2) * BLK : (mt % 2 + 1) * BLK]
                          for mt in range(MT)]
                else:
                    pp = [psU.tile([128, BLK], f32, tag="psU",
                                   name=f"pp_{wname}{j}_{t}")[:] for t in range(MT)]
                # one 2 MiB DMA per block streams at full HBM bandwidth
                xt = xs.tile([128, KT, BLK], f32r, tag="xs", name=f"xt_{wname}{j}")
                nc.sync.dma_start(
                    xt[:],
                    x_dram.ap()[:, j * BLK : (j + 1) * BLK]
                    .rearrange("(kt p) s -> p kt s", p=128),
                )
                for kt in range(KT):
                    for mt in range(MT):
                        nc.tensor.matmul(
                            pp[mt],
                            w_sb[:, kt, mt * 128 : (mt + 1) * 128],
                            xt[:, kt, :],
                            start=(kt == 0),
                            stop=(kt == KT - 1),
                            skip_group_check=True,
                        )
                for mt in range(MT):
                    nc.vector.tensor_scalar_add(
                        dst[:, mt, j * BLK : (j + 1) * BLK],
                        pp[mt],
                        bias_sb[:, mt : mt + 1],
                    )

        proj_qk(xk_d, wk_d, bk_sb, kt_, "k")
        proj_qk(xq_d, wq_d, bq_sb, qt, "q")

        # V projection: V[st*128+p, h*64+k] per Skv tile st, drained into vaug
        wv_sb = wpool.tile([128, KT, HK], f32r, tag="w")
        nc.sync.dma_start(wv_sb[:], wv_d.ap().rearrange("(kt p) m -> p kt m", p=128))
        for j in range(NB):
            xt = xs.tile([128, KT, BLK], f32r, tag="xs", name=f"xt_v{j}")
            nc.sync.dma_start(
                xt[:],
                xv_d.ap()[:, j * BLK : (j + 1) * BLK]
                .rearrange("(kt p) s -> p kt s", p=128),
            )
            # two passes of two q-subtiles each: V-proj only ever holds two
            # psU slots, so early attention (psu accumulators) isn't starved
            for qp in range(2):
                pp = [psU.tile([128, BLK], f32, tag="psU", name=f"pp_v{j}_{qp}_{t}")
                      for t in range(2)]
                for kt in range(KT):
                    for t in range(2):
                        q = 2 * qp + t
                        nc.tensor.matmul(
                            pp[t][:],
                            xt[:, kt, q * 128 : (q + 1) * 128],
                            wv_sb[:, kt, :],
                            start=(kt == 0),
                            stop=(kt == KT - 1),
                            skip_group_check=True,
                        )
                for t in range(2):
                    q = 2 * qp + t
                    st = j * 4 + q
                    nc.vector.tensor_copy(
                        vaug[:, st, :, 0:DK],
                        pp[t][:].rearrange("p (h k) -> p h k", h=LH),
                    )

        # Attention + output projection, fused per Sq-block
        wo_sb = wpool.tile([128, MT, D], f32r, tag="w")  # same slot bytes as proj weights
        nc.sync.dma_start(
            wo_sb[:], wo_d.ap().rearrange("(kt p) m -> p kt m", p=128)
        )
        def emit_outproj(jprev, ot_prev, mos):
            # output projection for block jprev, selected mo tiles; psy from the
            # psU pool so the scores (psS) pipeline keeps flowing
            for mo in mos:
                psy = psU.tile([128, BLK], f32, tag="psU", name=f"psy{jprev}_{mo}")
                for kt in range(MT):
                    nc.tensor.matmul(
                        psy[:],
                        wo_sb[:, kt, mo * 128 : (mo + 1) * 128],
                        ot_prev[:, kt, :],
                        start=(kt == 0),
                        stop=(kt == MT - 1),
                        skip_group_check=True,
                    )
                ysb = ypool.tile([128, BLK], f32, tag="y", name=f"ysb{jprev}_{mo}")
                nc.vector.tensor_copy(ysb[:], psy[:])
                nc.sync.dma_start(
                    y_d[mo * 128 : (mo + 1) * 128,
                        jprev * BLK : (jprev + 1) * BLK], ysb[:]
                )

        ot_prev = None
        for j in range(NB):
            otj = ot_pool.tile([128, MT, BLK], f32r)
            for hp in range(LH // 2):
                mt = hp
                psu = [psU.tile([DK + 1, BLK], f32, tag="psU",
                                name=f"ps_u{j}_{hp}_{p2}") for p2 in range(2)]
                for i in range(ST):
                    ps2 = psS.tile([128, 2 * BLK], f32, tag="psS",
                                   name=f"ps_s{j}_{hp}_{i}")
                    for pi in range(2):
                        bp = pi * 64
                        nc.tensor.matmul(
                            ps2[:, pi * BLK : (pi + 1) * BLK],
                            kt_[bp : bp + 64, mt, i * 128 : (i + 1) * 128],
                            qt[bp : bp + 64, mt, j * BLK : (j + 1) * BLK],
                            start=True,
                            stop=True,
                            skip_group_check=True,
                        )
                    es = es_pool.tile([128, 2 * BLK], bf16, tag="es")
                    nc.scalar.activation(es[:], ps2[:], Exp)
                    for pi in range(2):
                        h = 2 * hp + pi
                        nc.tensor.matmul(
                            psu[pi][:],
                            vaug[:, i, h, :],
                            es[:, pi * BLK : (pi + 1) * BLK],
                            start=(i == 0),
                            stop=(i == ST - 1),
                            skip_group_check=True,
                        )
                for pi in range(2):
                    bp = pi * 64
                    # r row (psum partition 64) -> partition 0 (plain DVE copy
                    # handles the shift; the custom reciprocal op does not, so
                    # it runs after, aligned at partition 0), then broadcast
                    # 1/r across 64 partitions on the idle GpSimd engine.
                    rrow = rpool.tile([1, BLK], f32, tag="r", name=f"rr{j}_{hp}_{pi}")
                    nc.vector.tensor_copy(rrow[:], psu[pi][DK : DK + 1, :])
                    rf = rpool.tile([1, BLK], f32, tag="rf", name=f"rf{j}_{hp}_{pi}")
                    nc.vector.reciprocal_approx_fast(rf[:], rrow[:])
                    rbc = upool.tile([DK, BLK], f32, tag="rb", name=f"rb{j}_{hp}_{pi}")
                    nc.gpsimd.partition_broadcast(rbc[:], rf[:])
                    nc.vector.tensor_mul(otj[bp : bp + 64, mt, :],
                                         psu[pi][0:DK, :], rbc[:])
                if hp == 0 and ot_prev is not None:
                    emit_outproj(j - 1, ot_prev, range(KT))
            ot_prev = otj
        emit_outproj(NB - 1, ot_prev, range(KT))

    nc.compile()
    return nc


def get_program():
    if "nc" not in _CACHE:
        _CACHE["nc"] = _build_program()
    return _CACHE["nc"]


def make_core_inputs(query, key, value, Wq, bq, Wk, bk, Wv, bv, Wo, bo):
    """Build the 8 per-core input dicts (and the folded output bias)."""
    f = np.float32
    in_maps = []
    for c in range(8):
        b, g = c // 2, c % 2
        hs = slice(g * LH, (g + 1) * LH)
        m = {
            "xq_t": np.ascontiguousarray(query[b].T, dtype=f),
            "xk_t": np.ascontiguousarray(key[b].T, dtype=f),
            "xv_t": np.ascontiguousarray(value[b].T, dtype=f),
            "wq": np.ascontiguousarray(
                Wq[hs].transpose(1, 0, 2).reshape(D, HK) / 8.0, dtype=f
            ),
            "wk": np.ascontiguousarray(
                Wk[hs].transpose(1, 0, 2).reshape(D, HK), dtype=f
            ),
            "wv": np.ascontiguousarray(
                Wv[hs].transpose(1, 0, 2).reshape(D, HK), dtype=f
            ),
            "wo": np.ascontiguousarray(Wo[g * HK : (g + 1) * HK, :], dtype=f),
            "bq2": np.ascontiguousarray(
                (bq[hs].reshape(HK) / 8.0).reshape(MT, 128).T, dtype=f
            ),
            "bk2": np.ascontiguousarray(
                bk[hs].reshape(HK).reshape(MT, 128).T, dtype=f
            ),
        }
        in_maps.append(m)
    bo_eff = (bv.reshape(H * DK).astype(np.float64) @ Wo.astype(np.float64)
              + bo.astype(np.float64)).astype(f)
    return in_maps, bo_eff


def combine_outputs(results, bo_eff):
    """results: list of 8 dicts with 'y_t' [D, S]. Returns [B, S, D] f32."""
    out = np.empty((B, S, D), dtype=np.float32)
    for b in range(B):
        acc = results[2 * b]["y_t"] + results[2 * b + 1]["y_t"]
        out[b] = acc.T + bo_eff[None, :]
    return out


def kernel(**inputs):
    from concourse.bass_utils import run_bass_kernel_spmd

    inputs = {k: np.asarray(v) for k, v in inputs.items()}
    nc = get_program()
    in_maps, bo_eff = make_core_inputs(
        inputs["query"], inputs["key"], inputs["value"],
        inputs["Wq"], inputs["bq"], inputs["Wk"], inputs["bk"],
        inputs["Wv"], inputs["bv"], inputs["Wo"], inputs["bo"],
    )
    res = run_bass_kernel_spmd(nc, in_maps, list(range(8)))
    return combine_outputs(res.results, bo_eff)
